# revision 1
# baseline (speedup 1.0000x reference)
"""Trainium2 Bass kernel for nn_AttentionDigitCaps (capsule dynamic routing).

reference math:
    x = inputs.reshape(B, N, iL)                      # B=32, N=2048, iL=32
    u = einsum('bji,jik->bjk', x, W).reshape(B,N,C,L) # C=L=32
    b = 0; for r in 3: c = softmax(b, C); s = sum_j u*c + biases; v = squash(s)
                       if r<2: b += sum_l u*v

Sharding: capsule dim N split over 8 cores (256 each) so the 256MB W is read
once per pass (33.5MB/core).  Collectives hang through the axon PJRT path, so
the three routing iterations run as THREE NEFF launches; the only cross-core
data is the partial s ([B,C,L] = 131KB/core), reduced on the host between
launches.  u is recomputed from W in each launch (a W re-stream costs the same
HBM traffic as re-reading a cached u would) and never materializes in HBM;
each launch's DVE/PE routing work is pipelined under its own W DMA stream.

Graph 1 (phase s0): s0 = (1/C) sum_j u  ==  (1/C) x_flat @ W_flat
    one big matmul contracting (j,i), K-tile = 128 rows = (4 capsules x 32 iL)
Graph 2 (one routing iteration, run twice):
    inputs: x, W, v_rep (v replicated to 128 partitions, host-prepped), b_in
    per 16-capsule group g (pipelined with the W DMA):
      einsum tiles (i,jcol) via tile_position -> psum[(jcol,b), (l,c')]
      evac (ACT) -> u_g bf16 [128, 4, 32, 32]
      binc = sum_l u*v  (DVE mult + pairwise tree over l, bf16 2x)
      b = b_in + binc ; c = softmax_c'(b)  (ACT exp + DVE)
      s_psum[32(b), (l,c')] += blockones.T @ (u*c)   (PE block-diag ones)
    outputs: s_partial, b_out
Host between launches: s = sum_cores(s_p) + bias; v = squash(s) (fp64).
"""

import os
import sys
import numpy as np

if "/opt/trn_rl_repo" not in sys.path:
    sys.path.insert(0, "/opt/trn_rl_repo")

CORES = 8
B, N, IL, C, L = 32, 2048, 32, 32, 32
NLOC = N // CORES          # 256 capsules per core
G = NLOC // 16             # 16 groups of 16 capsules
JH = NLOC // 4             # 64 j_hi values (4 capsules share each partition set)
CL = C * L                 # 1024
EPS = 1e-7

_CACHE = {}


def _mk_nc():
    from concourse import bacc
    return bacc.Bacc("TRN2", target_bir_lowering=False, debug=False,
                     num_devices=CORES)


def _common_params(nc, mybir):
    f32 = mybir.dt.float32
    x_p = nc.dram_tensor("x", [128, G, 4, B], f32, kind="ExternalInput")
    w_p = nc.dram_tensor("w", [G, 128, 4, CL], f32, kind="ExternalInput")
    return x_p, w_p


def _build_g1():
    """s0_partial = sum_j u (this core's j)  -> out [B, CL] f32."""
    from concourse import tile
    import concourse.mybir as mybir

    f32 = mybir.dt.float32
    AF = mybir.ActivationFunctionType

    nc = _mk_nc()
    x_p, w_p = _common_params(nc, mybir)
    s_out = nc.dram_tensor("sp", [B, CL], f32, kind="ExternalOutput")

    with tile.TileContext(nc) as tc:
        with (
            tc.tile_pool(name="const", bufs=1) as constp,
            tc.tile_pool(name="wstream", bufs=3) as wp,
            tc.tile_pool(name="acc", bufs=1, space="PSUM") as accp,
        ):
            x_sb = constp.tile([128, G, 4, B], f32)
            nc.sync.dma_start(out=x_sb[:], in_=x_p[:])
            s_ps = accp.tile([B, CL], f32, tag="sacc")
            kt = 0
            for g in range(G):
                w_t = wp.tile([128, 4, CL], f32, tag="w")
                nc.sync.dma_start(out=w_t[:], in_=w_p[g])
                for jc in range(4):
                    for h in range(2):
                        nc.tensor.matmul(
                            s_ps[:, 512 * h:512 * h + 512],
                            x_sb[:, g, jc, :],
                            w_t[:, jc, 512 * h:512 * h + 512],
                            start=(kt == 0), stop=(kt == G * 4 - 1),
                            skip_group_check=True)
                    kt += 1
            s_loc = constp.tile([B, CL], f32)
            nc.scalar.activation(s_loc[:], s_ps[:], AF.Copy)
            nc.sync.dma_start(out=s_out[:], in_=s_loc[:])

    nc.compile()
    return nc


def _build_g2():
    """One routing iteration: (x, W, v_rep, b_in) -> (s_partial, b_out)."""
    from concourse import tile
    import concourse.mybir as mybir

    f32 = mybir.dt.float32
    bf16 = mybir.dt.bfloat16
    AF = mybir.ActivationFunctionType
    OP = mybir.AluOpType
    AX = mybir.AxisListType

    nc = _mk_nc()
    w_p = nc.dram_tensor("w", [G, 128, 4, CL], f32, kind="ExternalInput")
    # block-diagonal x: xbd[(i,iL), g, jc, (cap,b)] = x[b, j(g,i,jc), iL]*d(cap==i)
    # -> ONE [K=128, M=128] matmul per (g, jc, h) instead of 16 tile-packed ones
    xbd_p = nc.dram_tensor("xbd", [128, G, 4, 128], f32, kind="ExternalInput")
    vrep_p = nc.dram_tensor("vrep", [128, CL], f32, kind="ExternalInput")
    bin_p = nc.dram_tensor("bin", [128, JH, C], f32, kind="ExternalInput")
    bones_p = nc.dram_tensor("blockones", [128, B], f32, kind="ExternalInput")
    s_out = nc.dram_tensor("sp", [B, CL], f32, kind="ExternalOutput")
    b_out = nc.dram_tensor("bout", [128, JH, C], f32, kind="ExternalOutput")

    with tile.TileContext(nc) as tc:
        with (
            tc.tile_pool(name="const", bufs=1) as constp,
            tc.tile_pool(name="wstream", bufs=3) as wp,
            tc.tile_pool(name="ug", bufs=2) as ugp,
            tc.tile_pool(name="work", bufs=1) as workp,
            tc.tile_pool(name="dwork", bufs=2) as dworkp,
            tc.tile_pool(name="eps", bufs=6, space="PSUM") as epsp,
            tc.tile_pool(name="acc", bufs=1, space="PSUM") as accp,
        ):
            x_sb = constp.tile([128, G, 4, 128], f32)
            vrep_f = constp.tile([128, CL], f32)
            v_rep = constp.tile([128, CL], bf16)
            b_sb = constp.tile([128, JH, C], f32)
            bones_f = constp.tile([128, B], f32)
            bones_bf = constp.tile([128, B], bf16)
            nc.sync.dma_start(out=x_sb[:], in_=xbd_p[:])
            nc.sync.dma_start(out=vrep_f[:], in_=vrep_p[:])
            nc.sync.dma_start(out=b_sb[:], in_=bin_p[:])
            nc.sync.dma_start(out=bones_f[:], in_=bones_p[:])
            nc.vector.tensor_copy(v_rep[:], vrep_f[:])
            nc.vector.tensor_copy(bones_bf[:], bones_f[:])

            s_ps = accp.tile([B, CL], f32, tag="sacc")
            JB = 8  # j_hi per chunk (2 W groups)
            vb = v_rep.rearrange("p (x l c) -> p x l c", x=1, c=C)
            vb = vb.broadcast_to([128, JB, L, C])

            for gg in range(G // 2):
                u_g = ugp.tile([128, JB, L, C], bf16, tag="ug")
                for g2 in range(2):
                    g = 2 * gg + g2
                    w_t = wp.tile([128, 4, CL], f32, tag="w")
                    nc.sync.dma_start(out=w_t[:], in_=w_p[g])
                    w_v = w_t.rearrange("p j (c l) -> p j c l", c=C)
                    for jc in range(4):
                        for h in range(2):
                            ps = epsp.tile([128, 512], f32, tag="eps")
                            rhs = w_v[:, jc, :, 16 * h:16 * h + 16]
                            rhs = rhs.rearrange("p c l -> p l c")
                            nc.tensor.matmul(ps[:], x_sb[:, g, jc, :], rhs,
                                             start=True, stop=True)
                            dst = u_g[:, 4 * g2 + jc, 16 * h:16 * h + 16, :]
                            nc.scalar.activation(
                                dst.rearrange("p l c -> p (l c)"), ps[:],
                                AF.Copy)

                # ---- binc = sum_l u*v ; b = b_in + binc ----------------
                t0 = workp.tile([128, JB, L, C], bf16, tag="t0")
                nc.vector.tensor_mul(t0[:], u_g[:], vb)
                t1 = workp.tile([128, JB, 16, C], bf16, tag="t1")
                nc.vector.tensor_add(t1[:], t0[:, :, 0:16, :],
                                     t0[:, :, 16:32, :])
                t2 = workp.tile([128, JB, 8, C], bf16, tag="t2")
                nc.vector.tensor_add(t2[:], t1[:, :, 0:8, :], t1[:, :, 8:16, :])
                t3 = workp.tile([128, JB, 4, C], bf16, tag="t3")
                nc.vector.tensor_add(t3[:], t2[:, :, 0:4, :], t2[:, :, 4:8, :])
                t4 = workp.tile([128, JB, 2, C], bf16, tag="t4")
                nc.vector.tensor_add(t4[:], t3[:, :, 0:2, :], t3[:, :, 2:4, :])
                b_c = b_sb[:, JB * gg:JB * gg + JB, :]
                t5 = workp.tile([128, JB, C], bf16, tag="t5")
                nc.vector.tensor_add(t5[:], t4[:, :, 0, :], t4[:, :, 1, :])
                nc.vector.tensor_add(b_c, b_c, t5[:])

                # ---- c = softmax_c'(b): exp+denominator on ACT ---------
                e = workp.tile([128, JB, C], bf16, tag="e")
                sE = workp.tile([128, JB], f32, tag="sE")
                for jj in range(JB):
                    nc.scalar.activation(e[:, jj, :], b_c[:, jj, :], AF.Exp,
                                         accum_out=sE[:, jj:jj + 1])
                rE = workp.tile([128, JB], f32, tag="rE")
                nc.vector.reciprocal(rE[:], sE[:])
                c_t = workp.tile([128, JB, C], bf16, tag="c")
                rE_b = rE.rearrange("p (j x) -> p j x", x=1)
                rE_b = rE_b.broadcast_to([128, JB, C])
                nc.vector.tensor_mul(c_t[:], e[:], rE_b)

                # ---- s_psum += blockones.T @ (u * c) -------------------
                c_b = c_t.rearrange("p j (x c) -> p j x c", x=1)
                c_b = c_b.broadcast_to([128, JB, L, C])
                tmp = dworkp.tile([128, JB, L, C], bf16, tag="tmp")
                nc.vector.tensor_mul(tmp[:], u_g[:], c_b)
                for kk in range(JB):
                    rhs = tmp[:, kk, :, :].rearrange("p l c -> p (l c)")
                    for hh in range(2):
                        nc.tensor.matmul(
                            s_ps[:, 512 * hh:512 * hh + 512],
                            bones_bf[:], rhs[:, 512 * hh:512 * hh + 512],
                            start=(gg == 0 and kk == 0),
                            stop=(gg == G // 2 - 1 and kk == JB - 1),
                            skip_group_check=True)

            s_loc = constp.tile([B, CL], f32)
            nc.scalar.activation(s_loc[:], s_ps[:], AF.Copy)
            nc.sync.dma_start(out=s_out[:], in_=s_loc[:])
            nc.sync.dma_start(out=b_out[:], in_=b_sb[:])

    nc.compile()
    return nc


def _host_prep(inputs, W):
    x = np.ascontiguousarray(inputs.reshape(B, N, IL), dtype=np.float32)
    W = np.ascontiguousarray(W, dtype=np.float32)
    # x shard: [r, (i,iL), g, jcol, b]
    xr = x.reshape(B, CORES, G, 4, 4, IL)
    x_sh = np.ascontiguousarray(
        xr.transpose(1, 3, 5, 2, 4, 0).reshape(CORES, 128, G, 4, B))
    # W shard: [r, g, (i,iL), jcol, cl]
    wr = W.reshape(CORES, G, 4, 4, IL, CL)
    w_sh = np.ascontiguousarray(
        wr.transpose(0, 1, 2, 4, 3, 5).reshape(CORES, G, 128, 4, CL))
    blockones = np.ascontiguousarray(
        np.tile(np.eye(B, dtype=np.float32), (4, 1)))
    # block-diagonal x for G2: xbd[r, (i,iL), g, jc, (cap,b)] nonzero iff cap==i
    xbd = np.zeros((CORES, 128, G, 4, 128), np.float32)
    for i in range(4):
        xbd[:, 32 * i:32 * i + 32, :, :, 32 * i:32 * i + 32] = \
            x_sh[:, 32 * i:32 * i + 32]
    return x_sh, w_sh, blockones, np.ascontiguousarray(xbd)


def _squash_np(s):
    """reference squash in float64; s is [B, C, L]."""
    s = s.astype(np.float64)
    n = np.linalg.norm(s, axis=-1, keepdims=True)
    return (n ** 2 / (1 + n ** 2) / (n + EPS)) * s


def _install_trace_hook():
    """Register the NTFF profiling hook (antenv.axon_hooks is absent in this
    container, but the ctypes implementation ships in trn_agent_boot)."""
    import types

    if "antenv.axon_hooks" in sys.modules:
        return
    try:
        from trn_agent_boot.trn_boot import _ntff_profile_via_ctypes
        hook = _ntff_profile_via_ctypes("/opt/axon/libaxon_pjrt.so")
        if hook is None:
            return
        m = types.ModuleType("antenv.axon_hooks")
        m.get_axon_ntff_profile_hook = lambda: hook
        sys.modules["antenv.axon_hooks"] = m
        from concourse import bass_utils
        bass_utils.upload_artifacts = lambda tmpdir: tmpdir  # no egress
    except Exception as e:  # profiling is best-effort
        print(f"trace hook install failed: {e}", file=sys.stderr)


def kernel(inputs, W, biases):
    from concourse.bass_utils import run_bass_kernel_spmd

    if "g1" not in _CACHE:
        _CACHE["g1"] = _build_g1()
        _CACHE["g2"] = _build_g2()
    g1, g2 = _CACHE["g1"], _CACHE["g2"]

    x_sh, w_sh, blockones, xbd = _host_prep(inputs, W)
    biases = np.asarray(biases, dtype=np.float64)
    trace = os.environ.get("KERNEL_TRACE", "0") == "1"
    if trace:
        _install_trace_hook()
    cores = list(range(CORES))
    results = []

    def launch(nc, maps):
        res = run_bass_kernel_spmd(nc, maps, core_ids=cores, trace=trace)
        results.append(res)
        return res.results

    # (l, c') flattened s <-> [C, L]: s_flat[b, l*C + c] = s[b, c, l]
    def s_from_flat(sp):  # [B, CL] -> [B, C, L]
        return sp.reshape(B, L, C).transpose(0, 2, 1)

    def vrep_from_v(v):   # v [B, C, L] -> [128, CL] f32 (l,c') order
        vf = np.ascontiguousarray(
            v.transpose(0, 2, 1).reshape(B, CL).astype(np.float32))
        return np.ascontiguousarray(np.tile(vf, (4, 1)))

    # --- launch 1: s0 (G1 psum cols are W's natural (c',l) order) -------
    r1 = launch(g1, [{"x": x_sh[r], "w": w_sh[r]} for r in cores])
    s0p = sum(np.asarray(r1[r]["sp"], np.float64) for r in cores)
    s0 = s0p.reshape(B, C, L) / C + biases
    v = _squash_np(s0)

    # --- launches 2,3: routing iterations -------------------------------
    b_in = [np.zeros((128, JH, C), np.float32) for _ in cores]
    for _ in range(2):
        vrep = vrep_from_v(v)
        r2 = launch(g2, [
            {"xbd": xbd[r], "w": w_sh[r], "vrep": vrep, "bin": b_in[r],
             "blockones": blockones} for r in cores])
        sp = sum(np.asarray(r2[r]["sp"], np.float64) for r in cores)
        s = s_from_flat(sp) + biases
        v = _squash_np(s)
        b_in = [np.asarray(r2[r]["bout"], np.float32) for r in cores]

    _CACHE["last_results"] = results
    return np.ascontiguousarray(v.astype(np.float32))



# revision 3
# speedup vs baseline: 1.3701x; 1.3701x over previous
"""Trainium2 Bass kernel for nn_AttentionDigitCaps (capsule dynamic routing).

reference math:
    x = inputs.reshape(B, N, iL)                      # B=32, N=2048, iL=32
    u = einsum('bji,jik->bjk', x, W).reshape(B,N,C,L) # C=L=32
    b = 0; for r in 3: c = softmax(b, C); s = sum_j u*c + biases; v = squash(s)
                       if r<2: b += sum_l u*v

Two launches (instead of one per routing iteration):

Launch A (capsule-sharded, 256 j per core): u = x @ W in bf16 streamed out
  to DRAM (16.8 MB/core) plus the s0 partial (sum_j u).  bf16 W halves the
  HBM traffic vs f32 and runs the PE at the full bf16 rate.
Host: reduce s0 across cores, v1 = squash(s0/C + bias) in f64, and
  all-to-all the u tensor from capsule-sharded to batch-sharded layout.
Launch B (batch-sharded, 4 b per core): all remaining routing math is
  batch-local, so BOTH remaining iterations run in one launch from
  SBUF-resident u (read once, 16.8 MB/core): b=sum_l u*v; c=softmax(b);
  s=sum_j c*u; v=squash(s) on-core; output v3 directly.

Launch B layout: partition p=(j32,b4) [j32 = an arbitrary 32-way capsule
  split, b4 = local batch], free dims (jj in 64, (c,l) with l inner).
  sum_l -> DVE tensor_reduce(X); softmax over c -> ACT exp + DVE reduce;
  sum_j -> PE matmul with a tiled eye(4) selector (contracts j32 over
  partitions, psum-accumulates over jj); v broadcast to 128 partitions via
  a tiny K=4 replicator matmul.
"""

import os
import sys
import numpy as np

if "/opt/trn_rl_repo" not in sys.path:
    sys.path.insert(0, "/opt/trn_rl_repo")

import ml_dtypes

BF16 = ml_dtypes.bfloat16

CORES = 8
B, N, IL, C, L = 32, 2048, 32, 32, 32
NLOC = N // CORES          # 256 capsules per core (launch A)
BLOC = B // CORES          # 4 batch items per core (launch B)
CL = C * L                 # 1024
JJ = N // 32               # 64 j-chunks in launch B
EPS = 1e-7

_CACHE = {}


def _mk_nc():
    from concourse import bacc
    return bacc.Bacc("TRN2", target_bir_lowering=False, debug=False,
                     num_devices=CORES)


def _build_A():
    """u[b, j_local, cl] (bf16, DRAM) + s0_partial = sum_j u  -> [B, CL] f32.

    j_local = c4*64 + g2*16 + a*4 + jc; partition for matmul K = (a, i),
    output partition M = (a, b).  xbd is the host-built block-diagonal x
    (zero where the a of K differs from the a of M) so one 128x128 matmul
    computes 4 capsules' per-capsule predictions at once.
    """
    from concourse import tile
    import concourse.mybir as mybir

    f32 = mybir.dt.float32
    bf16 = mybir.dt.bfloat16
    AF = mybir.ActivationFunctionType

    nc = _mk_nc()
    xbd_p = nc.dram_tensor("xbd", [128, 16, 4, 128], bf16, kind="ExternalInput")
    w_p = nc.dram_tensor("w", [4, 128, 16, CL], bf16, kind="ExternalInput")
    bones_p = nc.dram_tensor("bones", [128, B], bf16, kind="ExternalInput")
    u_out = nc.dram_tensor("u", [4, 128, 16, CL], bf16, kind="ExternalOutput")
    s0_out = nc.dram_tensor("s0", [B, CL], f32, kind="ExternalOutput")

    with tile.TileContext(nc) as tc:
        with (
            tc.tile_pool(name="const", bufs=1) as constp,
            tc.tile_pool(name="wstream", bufs=2) as wp,
            tc.tile_pool(name="ustream", bufs=2) as up,
            tc.tile_pool(name="eps", bufs=2, space="PSUM") as epsp,
            tc.tile_pool(name="acc", bufs=1, space="PSUM") as accp,
        ):
            xbd = constp.tile([128, 16, 4, 128], bf16)
            bones = constp.tile([128, B], bf16)
            nc.sync.dma_start(out=xbd[:], in_=xbd_p[:])
            nc.sync.dma_start(out=bones[:], in_=bones_p[:])

            s0_ps = accp.tile([B, CL], f32, tag="s0acc")
            for c4 in range(4):
                w_t = wp.tile([128, 16, CL], bf16, tag="w")
                nc.sync.dma_start(out=w_t[:], in_=w_p[c4])
                u_sb = up.tile([128, 16, CL], bf16, tag="u")
                for g2 in range(4):
                    for jc in range(4):
                        g = c4 * 4 + g2
                        m = g2 * 4 + jc
                        ps = epsp.tile([128, CL], f32, tag="ups")
                        for h in range(2):
                            nc.tensor.matmul(
                                ps[:, 512 * h:512 * h + 512],
                                xbd[:, g, jc, :],
                                w_t[:, m, 512 * h:512 * h + 512],
                                start=True, stop=True)
                        # evacuate psum -> bf16 SBUF, alternating engines
                        if m % 2 == 0:
                            nc.scalar.activation(u_sb[:, m, :], ps[:], AF.Copy)
                        else:
                            nc.vector.tensor_copy(u_sb[:, m, :], ps[:])
                # s0 partial: bones.T @ u (contracts a, keeps b) for the chunk
                for m in range(16):
                    for h in range(2):
                        nc.tensor.matmul(
                            s0_ps[:, 512 * h:512 * h + 512],
                            bones[:], u_sb[:, m, 512 * h:512 * h + 512],
                            start=(c4 == 0 and m == 0),
                            stop=(c4 == 3 and m == 15),
                            skip_group_check=True)
                nc.sync.dma_start(out=u_out[c4], in_=u_sb[:])

            s0_loc = constp.tile([B, CL], f32)
            nc.scalar.activation(s0_loc[:], s0_ps[:], AF.Copy)
            nc.sync.dma_start(out=s0_out[:], in_=s0_loc[:])

    nc.compile()
    return nc


def _build_B():
    """Routing iterations 1 and 2 for 4 local batch items, all capsules."""
    from concourse import tile
    import concourse.mybir as mybir

    f32 = mybir.dt.float32
    bf16 = mybir.dt.bfloat16
    AF = mybir.ActivationFunctionType
    OP = mybir.AluOpType
    AX = mybir.AxisListType

    nc = _mk_nc()
    u_p = nc.dram_tensor("u", [128, JJ, CL], bf16, kind="ExternalInput")
    vrep_p = nc.dram_tensor("vrep", [128, CL], bf16, kind="ExternalInput")
    selw_p = nc.dram_tensor("selw", [128, 4], bf16, kind="ExternalInput")
    repw_p = nc.dram_tensor("repw", [4, 128], bf16, kind="ExternalInput")
    bias4_p = nc.dram_tensor("bias4", [4, CL], f32, kind="ExternalInput")
    vout_p = nc.dram_tensor("vout", [4, CL], f32, kind="ExternalOutput")

    NK = 8          # jj-chunks
    KJ = JJ // NK   # jj per chunk

    with tile.TileContext(nc) as tc:
        with (
            tc.tile_pool(name="const", bufs=1) as constp,
            tc.tile_pool(name="ub", bufs=1) as ubp,
            tc.tile_pool(name="work", bufs=1) as workp,
            tc.tile_pool(name="small", bufs=1) as smallp,
            tc.tile_pool(name="sps", bufs=2, space="PSUM") as psp,
            tc.tile_pool(name="vps", bufs=1, space="PSUM") as vpsp,
        ):
            vrep1 = constp.tile([128, CL], bf16)
            selw = constp.tile([128, 4], bf16)
            repw = constp.tile([4, 128], bf16)
            bias4 = constp.tile([4, CL], f32)
            nc.sync.dma_start(out=vrep1[:], in_=vrep_p[:])
            nc.sync.dma_start(out=selw[:], in_=selw_p[:])
            nc.sync.dma_start(out=repw[:], in_=repw_p[:])
            nc.sync.dma_start(out=bias4[:], in_=bias4_p[:])

            u_tiles = []
            for k in range(NK):
                ut = ubp.tile([128, KJ, CL], bf16, tag=f"u{k}")
                nc.sync.dma_start(out=ut[:], in_=u_p[:, KJ * k:KJ * k + KJ, :])
                u_tiles.append(ut)

            b_state = constp.tile([128, JJ, C], f32)
            vcur = vrep1

            for it in range(2):
                s_ps = psp.tile([4, CL], f32, tag="sps")

                def flush(k, e):
                    """softmax tail + s accumulation for chunk k."""
                    z = workp.tile([128, KJ], f32, tag="z")
                    nc.vector.tensor_reduce(z[:], e[:], axis=AX.X, op=OP.add)
                    rz = workp.tile([128, KJ], f32, tag="rz")
                    nc.vector.reciprocal(rz[:], z[:])
                    cw = workp.tile([128, KJ, C], bf16, tag="cw")
                    rzb = rz.rearrange("p (j x) -> p j x", x=1)
                    rzb = rzb.broadcast_to([128, KJ, C])
                    nc.vector.tensor_mul(cw[:], e[:], rzb)
                    tmp = workp.tile([128, KJ, C, L], bf16, tag="tmp")
                    uc = u_tiles[k].rearrange("p j (c l) -> p j c l", c=C)
                    cwb = cw.rearrange("p j (c x) -> p j c x", x=1)
                    cwb = cwb.broadcast_to([128, KJ, C, L])
                    nc.vector.tensor_mul(tmp[:], uc, cwb)
                    for jj in range(KJ):
                        rhs = tmp[:, jj].rearrange("p c l -> p (c l)")
                        for h in range(2):
                            nc.tensor.matmul(
                                s_ps[:, 512 * h:512 * h + 512],
                                selw[:], rhs[:, 512 * h:512 * h + 512],
                                start=(k == 0 and jj == 0),
                                stop=(k == NK - 1 and jj == KJ - 1),
                                skip_group_check=True)

                carry = None
                for k in range(NK):
                    uc = u_tiles[k].rearrange("p j (c l) -> p j c l", c=C)
                    t0 = workp.tile([128, KJ, C, L], bf16, tag="t0")
                    vb = vcur.rearrange("p (x c l) -> p x c l", x=1, c=C)
                    vb = vb.broadcast_to([128, KJ, C, L])
                    nc.vector.tensor_mul(t0[:], uc, vb)
                    bc = b_state[:, KJ * k:KJ * k + KJ, :]
                    if it == 0:
                        nc.vector.tensor_reduce(bc, t0[:], axis=AX.X, op=OP.add)
                    else:
                        binc = workp.tile([128, KJ, C], f32, tag="binc")
                        nc.vector.tensor_reduce(binc[:], t0[:], axis=AX.X,
                                                op=OP.add)
                        nc.vector.tensor_add(bc, bc, binc[:])
                    e = workp.tile([128, KJ, C], bf16, tag="e", bufs=2)
                    nc.scalar.activation(e[:], bc, AF.Exp)
                    if carry is not None:
                        flush(*carry)
                    carry = (k, e)
                flush(*carry)

                # s = s_psum + bias; v = squash(s) on partitions 0..3
                s_sb = smallp.tile([4, CL], f32, tag="s")
                nc.vector.tensor_add(s_sb[:], s_ps[:], bias4[:])
                q2 = smallp.tile([4, CL], f32, tag="q2")
                nc.vector.tensor_mul(q2[:], s_sb[:], s_sb[:])
                qs = smallp.tile([4, C], f32, tag="qs")
                nc.vector.tensor_reduce(
                    qs[:], q2.rearrange("p (c l) -> p c l", c=C),
                    axis=AX.X, op=OP.add)
                nrm = smallp.tile([4, C], f32, tag="nrm")
                nc.scalar.activation(nrm[:], qs[:], AF.Sqrt)
                ne = smallp.tile([4, C], f32, tag="ne")
                nc.vector.tensor_scalar_add(ne[:], nrm[:], EPS)
                q1 = smallp.tile([4, C], f32, tag="q1")
                nc.vector.tensor_scalar_add(q1[:], qs[:], 1.0)
                den = smallp.tile([4, C], f32, tag="den")
                nc.vector.tensor_mul(den[:], ne[:], q1[:])
                rden = smallp.tile([4, C], f32, tag="rden")
                nc.vector.reciprocal(rden[:], den[:])
                fac = smallp.tile([4, C], f32, tag="fac")
                nc.vector.tensor_mul(fac[:], qs[:], rden[:])
                vn = smallp.tile([4, CL], f32, tag="vn")
                facb = fac.rearrange("p (c x) -> p c x", x=1)
                facb = facb.broadcast_to([4, C, L])
                nc.vector.tensor_mul(
                    vn.rearrange("p (c l) -> p c l", c=C),
                    s_sb.rearrange("p (c l) -> p c l", c=C), facb)

                if it == 0:
                    vb16 = smallp.tile([4, CL], bf16, tag="vb16")
                    nc.vector.tensor_copy(vb16[:], vn[:])
                    vps = vpsp.tile([128, CL], f32, tag="vrep")
                    for h in range(2):
                        nc.tensor.matmul(
                            vps[:, 512 * h:512 * h + 512],
                            repw[:], vb16[:, 512 * h:512 * h + 512],
                            start=True, stop=True)
                    vrep2 = constp.tile([128, CL], bf16)
                    nc.scalar.activation(vrep2[:], vps[:], AF.Copy)
                    vcur = vrep2
                else:
                    nc.sync.dma_start(out=vout_p[:], in_=vn[:])

    nc.compile()
    return nc


def _host_prep_A(inputs, W):
    """Build per-core bf16 inputs for launch A."""
    x = np.ascontiguousarray(inputs.reshape(B, N, IL), dtype=np.float32)
    # x_sh[r, (a,i), g, jc, b] = x[b, r*256+g*16+a*4+jc, i]
    xr = x.reshape(B, CORES, 16, 4, 4, IL)
    x_sh = xr.transpose(1, 3, 5, 2, 4, 0).reshape(CORES, 128, 16, 4, B)
    xbd = np.zeros((CORES, 128, 16, 4, 128), np.float32)
    for a in range(4):
        xbd[:, 32 * a:32 * a + 32, :, :, 32 * a:32 * a + 32] = \
            x_sh[:, 32 * a:32 * a + 32]
    xbd = np.ascontiguousarray(xbd).astype(BF16)
    # w_sh[r, c4, (a,i), (g2,jc), cl] = W[r*256+(c4*4+g2)*16+a*4+jc, i, cl]
    wr = np.asarray(W, np.float32).reshape(CORES, 4, 4, 4, 4, IL, CL)
    w_sh = np.ascontiguousarray(
        wr.transpose(0, 1, 3, 5, 2, 4, 6).reshape(CORES, 4, 128, 16, CL)
    ).astype(BF16)
    bones = np.ascontiguousarray(
        np.tile(np.eye(B, dtype=np.float32), (4, 1))).astype(BF16)
    return xbd, w_sh, bones


def _squash_np(s):
    """reference squash in float64; s is [B, C, L]."""
    s = s.astype(np.float64)
    n = np.linalg.norm(s, axis=-1, keepdims=True)
    return (n ** 2 / (1 + n ** 2) / (n + EPS)) * s


def _install_trace_hook():
    """Register the NTFF profiling hook (antenv.axon_hooks is absent in this
    container, but the ctypes implementation ships in trn_agent_boot)."""
    import types

    if "antenv.axon_hooks" in sys.modules:
        return
    try:
        from trn_agent_boot.trn_boot import _ntff_profile_via_ctypes
        hook = _ntff_profile_via_ctypes("/opt/axon/libaxon_pjrt.so")
        if hook is None:
            return
        m = types.ModuleType("antenv.axon_hooks")
        m.get_axon_ntff_profile_hook = lambda: hook
        sys.modules["antenv.axon_hooks"] = m
        from concourse import bass_utils
        bass_utils.upload_artifacts = lambda tmpdir: tmpdir  # no egress
    except Exception as e:  # profiling is best-effort
        print(f"trace hook install failed: {e}", file=sys.stderr)


def kernel(inputs, W, biases):
    from concourse.bass_utils import run_bass_kernel_spmd

    if "ga" not in _CACHE:
        _CACHE["ga"] = _build_A()
        _CACHE["gb"] = _build_B()
    ga, gb = _CACHE["ga"], _CACHE["gb"]

    xbd, w_sh, bones = _host_prep_A(inputs, W)
    biases64 = np.asarray(biases, dtype=np.float64)
    trace = os.environ.get("KERNEL_TRACE", "0") == "1"
    if trace:
        _install_trace_hook()
    cores = list(range(CORES))
    results = []

    def launch(nc, maps):
        res = run_bass_kernel_spmd(nc, maps, core_ids=cores, trace=trace)
        results.append(res)
        return res.results

    # --- launch A: u (bf16) + s0 partials --------------------------------
    rA = launch(ga, [{"xbd": xbd[r], "w": w_sh[r], "bones": bones}
                     for r in cores])
    s0 = sum(np.asarray(rA[r]["s0"], np.float64) for r in cores)
    v1 = _squash_np(s0.reshape(B, C, L) / C + biases64)

    # --- host: capsule-shard -> batch-shard all-to-all of u --------------
    # u_a[q] is [c4, (a,b), (g2,jc), cl]; local j = c4*64 + g2*16 + a*4 + jc
    blocks = []
    for q in cores:
        ua = np.asarray(rA[q]["u"]).view(np.uint16)
        ua = ua.reshape(4, 4, 32, 4, 4, CL).transpose(0, 3, 1, 4, 2, 5)
        blocks.append(ua.reshape(NLOC, B, CL))
    U = np.concatenate(blocks, axis=0)          # [N, B, CL] (uint16 view)
    U = U.reshape(JJ, 32, B, CL)                # [jj, j32, b, cl]

    v1f = np.ascontiguousarray(v1.reshape(B, CL).astype(np.float32))
    selw = np.ascontiguousarray(
        np.tile(np.eye(4, dtype=np.float32), (32, 1))).astype(BF16)
    repw = np.ascontiguousarray(
        np.tile(np.eye(4, dtype=np.float32), (1, 32))).astype(BF16)
    bias4 = np.ascontiguousarray(
        np.tile(np.asarray(biases, np.float32).reshape(1, CL), (4, 1)))

    mapsB = []
    for r in cores:
        ub = np.ascontiguousarray(
            U[:, :, 4 * r:4 * r + 4, :].transpose(1, 2, 0, 3)
            .reshape(128, JJ, CL)).view(BF16)
        vrep = np.ascontiguousarray(
            np.tile(v1f[4 * r:4 * r + 4], (32, 1))).astype(BF16)
        mapsB.append({"u": ub, "vrep": vrep, "selw": selw, "repw": repw,
                      "bias4": bias4})

    # --- launch B: routing iterations 1+2, batch-local -------------------
    rB = launch(gb, mapsB)
    v = np.empty((B, C, L), np.float32)
    for r in cores:
        v[4 * r:4 * r + 4] = np.asarray(rB[r]["vout"],
                                        np.float32).reshape(BLOC, C, L)

    _CACHE["last_results"] = results
    return np.ascontiguousarray(v)


# revision 5
# speedup vs baseline: 1.8242x; 1.3314x over previous
"""Trainium2 Bass kernel for nn_AttentionDigitCaps (capsule dynamic routing).

reference math:
    x = inputs.reshape(B, N, iL)                      # B=32, N=2048, iL=32
    u = einsum('bji,jik->bjk', x, W).reshape(B,N,C,L) # C=L=32
    b = 0; for r in 3: c = softmax(b, C); s = sum_j u*c + biases; v = squash(s)
                       if r<2: b += sum_l u*v

Two launches (instead of one per routing iteration):

Launch A (capsule-sharded, 256 j per core): u = x @ W in bf16 streamed out
  to DRAM (16.8 MB/core) plus the s0 partial (sum_j u).  bf16 W halves the
  HBM traffic vs f32 and runs the PE at the full bf16 rate.
Host: reduce s0 across cores, v1 = squash(s0/C + bias) in f64, and
  all-to-all the u tensor from capsule-sharded to batch-sharded layout.
Launch B (batch-sharded, 4 b per core): all remaining routing math is
  batch-local, so BOTH remaining iterations run in one launch from
  SBUF-resident u (read once, 16.8 MB/core): b=sum_l u*v; c=softmax(b);
  s=sum_j c*u; v=squash(s) on-core; output v3 directly.

Launch B layout: partition p=(j32,b4) [j32 = an arbitrary 32-way capsule
  split, b4 = local batch], free dims (jj in 64, (c,l) with l inner).
  sum_l -> DVE tensor_reduce(X); softmax over c -> ACT exp + DVE reduce;
  sum_j -> PE matmul with a tiled eye(4) selector (contracts j32 over
  partitions, psum-accumulates over jj); v broadcast to 128 partitions via
  a tiny K=4 replicator matmul.
"""

import os
import sys
import numpy as np

if "/opt/trn_rl_repo" not in sys.path:
    sys.path.insert(0, "/opt/trn_rl_repo")

import ml_dtypes

BF16 = ml_dtypes.bfloat16

CORES = 8
B, N, IL, C, L = 32, 2048, 32, 32, 32
NLOC = N // CORES          # 256 capsules per core (launch A)
BLOC = B // CORES          # 4 batch items per core (launch B)
CL = C * L                 # 1024
JJ = N // 32               # 64 j-chunks in launch B
EPS = 1e-7

_CACHE = {}


def _mk_nc():
    from concourse import bacc
    return bacc.Bacc("TRN2", target_bir_lowering=False, debug=False,
                     num_devices=CORES)


def _build_A():
    """u[b, j_local, cl] (bf16, DRAM) + s0_partial = sum_j u  -> [B, CL] f32.

    j_local = c4*64 + g2*16 + a*4 + jc; partition for matmul K = (a, i),
    output partition M = (a, b).  xbd is the host-built block-diagonal x
    (zero where the a of K differs from the a of M) so one 128x128 matmul
    computes 4 capsules' per-capsule predictions at once.
    """
    from concourse import tile
    import concourse.mybir as mybir

    f32 = mybir.dt.float32
    bf16 = mybir.dt.bfloat16
    AF = mybir.ActivationFunctionType

    nc = _mk_nc()
    xbd_p = nc.dram_tensor("xbd", [128, 16, 4, 128], bf16, kind="ExternalInput")
    w_p = nc.dram_tensor("w", [4, 128, 16, CL], bf16, kind="ExternalInput")
    bones_p = nc.dram_tensor("bones", [128, B], bf16, kind="ExternalInput")
    u_out = nc.dram_tensor("u", [4, 128, 16, CL], bf16, kind="ExternalOutput")
    s0_out = nc.dram_tensor("s0", [B, CL], f32, kind="ExternalOutput")

    with tile.TileContext(nc) as tc:
        with (
            tc.tile_pool(name="const", bufs=1) as constp,
            tc.tile_pool(name="wstream", bufs=2) as wp,
            tc.tile_pool(name="ustream", bufs=2) as up,
            tc.tile_pool(name="eps", bufs=2, space="PSUM") as epsp,
            tc.tile_pool(name="acc", bufs=1, space="PSUM") as accp,
        ):
            xbd = constp.tile([128, 16, 4, 128], bf16)
            bones = constp.tile([128, B], bf16)
            nc.sync.dma_start(out=xbd[:], in_=xbd_p[:])
            nc.sync.dma_start(out=bones[:], in_=bones_p[:])

            s0_ps = accp.tile([B, CL], f32, tag="s0acc")
            for c4 in range(4):
                w_t = wp.tile([128, 16, CL], bf16, tag="w")
                nc.sync.dma_start(out=w_t[:], in_=w_p[c4])
                u_sb = up.tile([128, 16, CL], bf16, tag="u")
                for g2 in range(4):
                    for jc in range(4):
                        g = c4 * 4 + g2
                        m = g2 * 4 + jc
                        ps = epsp.tile([128, CL], f32, tag="ups")
                        for h in range(2):
                            nc.tensor.matmul(
                                ps[:, 512 * h:512 * h + 512],
                                xbd[:, g, jc, :],
                                w_t[:, m, 512 * h:512 * h + 512],
                                start=True, stop=True)
                        # evacuate psum -> bf16 SBUF, alternating engines
                        if m % 2 == 0:
                            nc.scalar.activation(u_sb[:, m, :], ps[:], AF.Copy)
                        else:
                            nc.vector.tensor_copy(u_sb[:, m, :], ps[:])
                # s0 partial: bones.T @ u (contracts a, keeps b) for the chunk
                for m in range(16):
                    for h in range(2):
                        nc.tensor.matmul(
                            s0_ps[:, 512 * h:512 * h + 512],
                            bones[:], u_sb[:, m, 512 * h:512 * h + 512],
                            start=(c4 == 0 and m == 0),
                            stop=(c4 == 3 and m == 15),
                            skip_group_check=True)
                nc.sync.dma_start(out=u_out[c4], in_=u_sb[:])

            s0_loc = constp.tile([B, CL], f32)
            nc.scalar.activation(s0_loc[:], s0_ps[:], AF.Copy)
            nc.sync.dma_start(out=s0_out[:], in_=s0_loc[:])

    nc.compile()
    return nc


def _build_B():
    """Routing iterations 1 and 2 for 4 local batch items, all capsules."""
    from concourse import tile
    import concourse.mybir as mybir

    f32 = mybir.dt.float32
    bf16 = mybir.dt.bfloat16
    AF = mybir.ActivationFunctionType
    OP = mybir.AluOpType
    AX = mybir.AxisListType

    nc = _mk_nc()
    u_p = nc.dram_tensor("u", [128, JJ, CL], bf16, kind="ExternalInput")
    vrep_p = nc.dram_tensor("vrep", [128, CL], bf16, kind="ExternalInput")
    selw_p = nc.dram_tensor("selw", [128, 4], bf16, kind="ExternalInput")
    repw_p = nc.dram_tensor("repw", [4, 128], bf16, kind="ExternalInput")
    bias4_p = nc.dram_tensor("bias4", [4, CL], f32, kind="ExternalInput")
    vout_p = nc.dram_tensor("vout", [4, CL], f32, kind="ExternalOutput")

    NK = 8          # jj-chunks
    KJ = JJ // NK   # jj per chunk

    with tile.TileContext(nc) as tc:
        with (
            tc.tile_pool(name="const", bufs=1) as constp,
            tc.tile_pool(name="ub", bufs=1) as ubp,
            tc.tile_pool(name="work", bufs=1) as workp,
            tc.tile_pool(name="small", bufs=1) as smallp,
            tc.tile_pool(name="sps", bufs=2, space="PSUM") as psp,
            tc.tile_pool(name="vps", bufs=1, space="PSUM") as vpsp,
        ):
            vrep1 = constp.tile([128, CL], bf16)
            selw = constp.tile([128, 4], bf16)
            repw = constp.tile([4, 128], bf16)
            bias4 = constp.tile([4, CL], f32)
            nc.sync.dma_start(out=vrep1[:], in_=vrep_p[:])
            nc.sync.dma_start(out=selw[:], in_=selw_p[:])
            nc.sync.dma_start(out=repw[:], in_=repw_p[:])
            nc.sync.dma_start(out=bias4[:], in_=bias4_p[:])

            u_tiles = []
            for k in range(NK):
                ut = ubp.tile([128, KJ, CL], bf16, tag=f"u{k}")
                nc.sync.dma_start(out=ut[:], in_=u_p[:, KJ * k:KJ * k + KJ, :])
                u_tiles.append(ut)

            b_state = constp.tile([128, JJ, C], f32)
            vcur = vrep1

            for it in range(2):
                s_ps = psp.tile([4, CL], f32, tag="sps")

                def flush(k, e):
                    """softmax tail + s accumulation for chunk k."""
                    z = workp.tile([128, KJ], f32, tag="z")
                    nc.vector.tensor_reduce(z[:], e[:], axis=AX.X, op=OP.add)
                    rz = workp.tile([128, KJ], f32, tag="rz")
                    nc.vector.reciprocal(rz[:], z[:])
                    cw = workp.tile([128, KJ, C], bf16, tag="cw")
                    rzb = rz.rearrange("p (j x) -> p j x", x=1)
                    rzb = rzb.broadcast_to([128, KJ, C])
                    nc.vector.tensor_mul(cw[:], e[:], rzb)
                    tmp = workp.tile([128, KJ, L, C], bf16, tag="tmp")
                    uc = u_tiles[k].rearrange("p j (l c) -> p j l c", l=L)
                    cwb = cw.rearrange("p j (x c) -> p j x c", x=1)
                    cwb = cwb.broadcast_to([128, KJ, L, C])
                    nc.vector.tensor_mul(tmp[:], uc, cwb)
                    for jj in range(KJ):
                        rhs = tmp[:, jj].rearrange("p c l -> p (c l)")
                        for h in range(2):
                            nc.tensor.matmul(
                                s_ps[:, 512 * h:512 * h + 512],
                                selw[:], rhs[:, 512 * h:512 * h + 512],
                                start=(k == 0 and jj == 0),
                                stop=(k == NK - 1 and jj == KJ - 1),
                                skip_group_check=True)

                carry = None
                for k in range(NK):
                    uc = u_tiles[k].rearrange("p j (l c) -> p j l c", l=L)
                    t0 = workp.tile([128, KJ, L, C], bf16, tag="t0")
                    vb = vcur.rearrange("p (x l c) -> p x l c", x=1, l=L)
                    vb = vb.broadcast_to([128, KJ, L, C])
                    nc.vector.tensor_mul(t0[:], uc, vb)
                    # sum over l: in-place bf16 add-tree, c contiguous
                    for hw in (16, 8, 4, 2):
                        nc.vector.tensor_add(t0[:, :, 0:hw, :],
                                             t0[:, :, 0:hw, :],
                                             t0[:, :, hw:2 * hw, :])
                    bc = b_state[:, KJ * k:KJ * k + KJ, :]
                    if it == 0:
                        nc.vector.tensor_add(bc, t0[:, :, 0, :], t0[:, :, 1, :])
                    else:
                        r5 = workp.tile([128, KJ, C], bf16, tag="r5")
                        nc.vector.tensor_add(r5[:], t0[:, :, 0, :],
                                             t0[:, :, 1, :])
                        nc.vector.tensor_add(bc, bc, r5[:])
                    e = workp.tile([128, KJ, C], bf16, tag="e", bufs=2)
                    nc.scalar.activation(e[:], bc, AF.Exp)
                    if carry is not None:
                        flush(*carry)
                    carry = (k, e)
                flush(*carry)

                # s = s_psum + bias; v = squash(s) on partitions 0..3
                s_sb = smallp.tile([4, CL], f32, tag="s")
                nc.vector.tensor_add(s_sb[:], s_ps[:], bias4[:])
                q2 = smallp.tile([4, CL], f32, tag="q2")
                nc.vector.tensor_mul(q2[:], s_sb[:], s_sb[:])
                q2v = q2.rearrange("p (l c) -> p l c", l=L)
                for hw in (16, 8, 4, 2):
                    nc.vector.tensor_add(q2v[:, 0:hw, :], q2v[:, 0:hw, :],
                                         q2v[:, hw:2 * hw, :])
                qs = smallp.tile([4, C], f32, tag="qs")
                nc.vector.tensor_add(qs[:], q2v[:, 0, :], q2v[:, 1, :])
                nrm = smallp.tile([4, C], f32, tag="nrm")
                nc.scalar.activation(nrm[:], qs[:], AF.Sqrt)
                ne = smallp.tile([4, C], f32, tag="ne")
                nc.vector.tensor_scalar_add(ne[:], nrm[:], EPS)
                q1 = smallp.tile([4, C], f32, tag="q1")
                nc.vector.tensor_scalar_add(q1[:], qs[:], 1.0)
                den = smallp.tile([4, C], f32, tag="den")
                nc.vector.tensor_mul(den[:], ne[:], q1[:])
                rden = smallp.tile([4, C], f32, tag="rden")
                nc.vector.reciprocal(rden[:], den[:])
                fac = smallp.tile([4, C], f32, tag="fac")
                nc.vector.tensor_mul(fac[:], qs[:], rden[:])
                vn = smallp.tile([4, CL], f32, tag="vn")
                facb = fac.rearrange("p (x c) -> p x c", x=1)
                facb = facb.broadcast_to([4, L, C])
                nc.vector.tensor_mul(
                    vn.rearrange("p (l c) -> p l c", l=L),
                    s_sb.rearrange("p (l c) -> p l c", l=L), facb)

                if it == 0:
                    vb16 = smallp.tile([4, CL], bf16, tag="vb16")
                    nc.vector.tensor_copy(vb16[:], vn[:])
                    vps = vpsp.tile([128, CL], f32, tag="vrep")
                    for h in range(2):
                        nc.tensor.matmul(
                            vps[:, 512 * h:512 * h + 512],
                            repw[:], vb16[:, 512 * h:512 * h + 512],
                            start=True, stop=True)
                    vrep2 = constp.tile([128, CL], bf16)
                    nc.scalar.activation(vrep2[:], vps[:], AF.Copy)
                    vcur = vrep2
                else:
                    nc.sync.dma_start(out=vout_p[:], in_=vn[:])

    nc.compile()
    return nc


def _host_prep_A(inputs, W):
    """Build per-core bf16 inputs for launch A."""
    x = np.ascontiguousarray(inputs.reshape(B, N, IL), dtype=np.float32)
    # x_sh[r, (a,i), g, jc, b] = x[b, r*256+g*16+a*4+jc, i]
    xr = x.reshape(B, CORES, 16, 4, 4, IL)
    x_sh = xr.transpose(1, 3, 5, 2, 4, 0).reshape(CORES, 128, 16, 4, B)
    xbd = np.zeros((CORES, 128, 16, 4, 128), np.float32)
    for a in range(4):
        xbd[:, 32 * a:32 * a + 32, :, :, 32 * a:32 * a + 32] = \
            x_sh[:, 32 * a:32 * a + 32]
    xbd = np.ascontiguousarray(xbd).astype(BF16)
    # w_sh[r, c4, (a,i), (g2,jc), cl] = W[r*256+(c4*4+g2)*16+a*4+jc, i, cl]
    wr = np.asarray(W, np.float32).reshape(CORES, 4, 4, 4, 4, IL, C, L)
    w_sh = np.ascontiguousarray(
        wr.transpose(0, 1, 3, 5, 2, 4, 7, 6).reshape(CORES, 4, 128, 16, CL)
    ).astype(BF16)
    bones = np.ascontiguousarray(
        np.tile(np.eye(B, dtype=np.float32), (4, 1))).astype(BF16)
    return xbd, w_sh, bones


def _squash_np(s):
    """reference squash in float64; s is [B, C, L]."""
    s = s.astype(np.float64)
    n = np.linalg.norm(s, axis=-1, keepdims=True)
    return (n ** 2 / (1 + n ** 2) / (n + EPS)) * s


def _install_trace_hook():
    """Register the NTFF profiling hook (antenv.axon_hooks is absent in this
    container, but the ctypes implementation ships in trn_agent_boot)."""
    import types

    if "antenv.axon_hooks" in sys.modules:
        return
    try:
        from trn_agent_boot.trn_boot import _ntff_profile_via_ctypes
        hook = _ntff_profile_via_ctypes("/opt/axon/libaxon_pjrt.so")
        if hook is None:
            return
        m = types.ModuleType("antenv.axon_hooks")
        m.get_axon_ntff_profile_hook = lambda: hook
        sys.modules["antenv.axon_hooks"] = m
        from concourse import bass_utils
        bass_utils.upload_artifacts = lambda tmpdir: tmpdir  # no egress
    except Exception as e:  # profiling is best-effort
        print(f"trace hook install failed: {e}", file=sys.stderr)


def kernel(inputs, W, biases):
    from concourse.bass_utils import run_bass_kernel_spmd

    if "ga" not in _CACHE:
        _CACHE["ga"] = _build_A()
        _CACHE["gb"] = _build_B()
    ga, gb = _CACHE["ga"], _CACHE["gb"]

    xbd, w_sh, bones = _host_prep_A(inputs, W)
    biases64 = np.asarray(biases, dtype=np.float64)
    trace = os.environ.get("KERNEL_TRACE", "0") == "1"
    if trace:
        _install_trace_hook()
    cores = list(range(CORES))
    results = []

    def launch(nc, maps):
        res = run_bass_kernel_spmd(nc, maps, core_ids=cores, trace=trace)
        results.append(res)
        return res.results

    # --- launch A: u (bf16) + s0 partials --------------------------------
    rA = launch(ga, [{"xbd": xbd[r], "w": w_sh[r], "bones": bones}
                     for r in cores])
    s0 = sum(np.asarray(rA[r]["s0"], np.float64) for r in cores)
    v1 = _squash_np(s0.reshape(B, L, C).transpose(0, 2, 1) / C + biases64)

    # --- host: capsule-shard -> batch-shard all-to-all of u --------------
    # u_a[q] is [c4, (a,b), (g2,jc), cl]; local j = c4*64 + g2*16 + a*4 + jc
    blocks = []
    for q in cores:
        ua = np.asarray(rA[q]["u"]).view(np.uint16)
        ua = ua.reshape(4, 4, 32, 4, 4, CL).transpose(0, 3, 1, 4, 2, 5)
        blocks.append(ua.reshape(NLOC, B, CL))
    U = np.concatenate(blocks, axis=0)          # [N, B, CL] (uint16 view)
    U = U.reshape(JJ, 32, B, CL)                # [jj, j32, b, cl]

    v1f = np.ascontiguousarray(
        v1.transpose(0, 2, 1).reshape(B, CL).astype(np.float32))
    selw = np.ascontiguousarray(
        np.tile(np.eye(4, dtype=np.float32), (32, 1))).astype(BF16)
    repw = np.ascontiguousarray(
        np.tile(np.eye(4, dtype=np.float32), (1, 32))).astype(BF16)
    bias4 = np.ascontiguousarray(
        np.tile(np.asarray(biases, np.float32).T.reshape(1, CL), (4, 1)))

    mapsB = []
    for r in cores:
        ub = np.ascontiguousarray(
            U[:, :, 4 * r:4 * r + 4, :].transpose(1, 2, 0, 3)
            .reshape(128, JJ, CL)).view(BF16)
        vrep = np.ascontiguousarray(
            np.tile(v1f[4 * r:4 * r + 4], (32, 1))).astype(BF16)
        mapsB.append({"u": ub, "vrep": vrep, "selw": selw, "repw": repw,
                      "bias4": bias4})

    # --- launch B: routing iterations 1+2, batch-local -------------------
    rB = launch(gb, mapsB)
    v = np.empty((B, C, L), np.float32)
    for r in cores:
        v[4 * r:4 * r + 4] = np.asarray(rB[r]["vout"], np.float32).reshape(
            BLOC, L, C).transpose(0, 2, 1)

    _CACHE["last_results"] = results
    return np.ascontiguousarray(v)
